# revision 2
# baseline (speedup 1.0000x reference)
"""Trainium2 Bass kernel for Bahdanau-style attention (nn_Attention).

Reference computation (B=128, S=1024, D=512):
    proj = tanh(concat(dec, enc) @ W1.T + b1)        # [B, S, D]
    scores = proj @ W2.T (+ b2, cancels in softmax)  # [B, S]
    alpha = softmax(scores, axis=1)
    context = einsum('bs,bsd->bd', alpha, enc)       # [B, D]

Strategy: pure data-parallel over batch (16 rows per NeuronCore, 8 cores).
Per-core dataflow (all matmuls bf16, fp32 PSUM accumulate):
  - hiddenT layout [h, s]: stationary = W1enc^T chunks, moving = enc^T tiles,
    so (proj_dec[b] + b1) becomes a per-partition bias fused into the
    ScalarE tanh that evacuates PSUM.
  - scores = W2 . hiddenT via PE matmuls with W2 column chunks as stationary.
  - softmax batched over groups of 4 batch rows on DVE/ScalarE
    (Exp with bias=-max and fused accum_out for the denominator).
  - alpha (normalized, bf16) transposed via PE transpose; context = alphaT^T @
    enc_natural via PE matmuls.
Host side: shard batch, pre-transpose/cast enc to both layouts in bf16.
"""

import numpy as np
import ml_dtypes

B, S, D = 128, 1024, 512
N_CORES = 8
B_LOC = B // N_CORES          # 16
GB = 4                        # batch rows per softmax group
NG = B_LOC // GB              # 4 groups
DC = D // 128                 # 4 chunks of 128 along d (and h)
SBLK = 512                    # s block for proj/score tiles
NSB = S // SBLK               # 2
NSC = S // 128                # 8 s-chunks of 128

_NPBF = ml_dtypes.bfloat16
_CACHE: dict = {}


def _build():
    from contextlib import ExitStack
    import concourse.bass as bass  # noqa: F401
    import concourse.tile as tile
    from concourse import bacc, mybir

    f32, bf16 = mybir.dt.float32, mybir.dt.bfloat16
    AX = mybir.AxisListType
    OP = mybir.AluOpType
    AF = mybir.ActivationFunctionType

    nc = bacc.Bacc("TRN2", target_bir_lowering=False, debug=False,
                   num_devices=N_CORES)

    encT = nc.dram_tensor("encT", [DC, 128, B_LOC, S], bf16, kind="ExternalInput").ap()
    encN = nc.dram_tensor("encN", [B_LOC, NSC, 128, D], bf16, kind="ExternalInput").ap()
    w1eT = nc.dram_tensor("w1eT", [DC, 128, D], bf16, kind="ExternalInput").ap()
    w1dT = nc.dram_tensor("w1dT", [DC, 128, D], bf16, kind="ExternalInput").ap()
    decT = nc.dram_tensor("decT", [DC, 128, B_LOC], bf16, kind="ExternalInput").ap()
    b1c = nc.dram_tensor("b1c", [DC, 128, 1], f32, kind="ExternalInput").ap()
    w2c = nc.dram_tensor("w2c", [DC, 128, 1], bf16, kind="ExternalInput").ap()
    ident = nc.dram_tensor("ident", [128, 128], bf16, kind="ExternalInput").ap()
    out = nc.dram_tensor("out", [B_LOC, D], f32, kind="ExternalOutput").ap()

    with tile.TileContext(nc) as tc, ExitStack() as ctx:
        singles = ctx.enter_context(tc.tile_pool(name="singles", bufs=1))
        w1e_sb = singles.tile([128, DC, D], bf16)
        nc.sync.dma_start(out=w1e_sb, in_=w1eT.rearrange("dc p h -> p dc h"))
        w1d_sb = singles.tile([128, DC, D], bf16)
        nc.sync.dma_start(out=w1d_sb, in_=w1dT.rearrange("dc p h -> p dc h"))
        dec_sb = singles.tile([128, DC, B_LOC], bf16)
        nc.sync.dma_start(out=dec_sb, in_=decT.rearrange("dc p b -> p dc b"))
        b1_sb = singles.tile([128, DC, 1], f32)
        nc.sync.dma_start(out=b1_sb, in_=b1c.rearrange("dc p o -> p dc o"))
        w2_sb = singles.tile([128, DC, 1], bf16)
        nc.sync.dma_start(out=w2_sb, in_=w2c.rearrange("dc p o -> p dc o"))
        ident_sb = singles.tile([128, 128], bf16)
        nc.sync.dma_start(out=ident_sb, in_=ident)
        pdb1 = singles.tile([128, DC, B_LOC], f32)

        # proj_decT[h, b] + b1[h], with h on partitions (4 chunks).
        with tc.tile_pool(name="pdps", bufs=1, space="PSUM") as pdps:
            for hc in range(DC):
                pd_ps = pdps.tile([128, B_LOC], f32, tag="pd")
                for dc in range(DC):
                    nc.tensor.matmul(
                        pd_ps,
                        lhsT=w1d_sb[:, dc, hc * 128:(hc + 1) * 128],
                        rhs=dec_sb[:, dc, :],
                        start=(dc == 0), stop=(dc == DC - 1))
                nc.scalar.activation(out=pdb1[:, hc, :], in_=pd_ps,
                                     func=AF.Identity, bias=b1_sb[:, hc, :],
                                     scale=1.0)

        encT_pool = ctx.enter_context(tc.tile_pool(name="encTp", bufs=3))
        encN_pool = ctx.enter_context(tc.tile_pool(name="encNp", bufs=GB + 2))
        hT_pool = ctx.enter_context(tc.tile_pool(name="hTp", bufs=3))
        sg_pool = ctx.enter_context(tc.tile_pool(name="sgp", bufs=2))
        small = ctx.enter_context(tc.tile_pool(name="small", bufs=2))
        at_pool = ctx.enter_context(tc.tile_pool(name="atp", bufs=2))
        ctxg_pool = ctx.enter_context(tc.tile_pool(name="ctxgp", bufs=2))
        ph_pool = ctx.enter_context(tc.tile_pool(name="php", bufs=5, space="PSUM"))
        misc_ps = ctx.enter_context(tc.tile_pool(name="miscps", bufs=3, space="PSUM"))

        for g in range(NG):
            # group rows live at partitions {0, 32, 64, 96}: engine writes to
            # a single partition are only legal at 32-aligned bases.
            scores_g = sg_pool.tile([128, S], f32, tag="scores")
            encN_bs = []
            for bi in range(GB):
                b = g * GB + bi
                encT_b = encT_pool.tile([128, DC, S], bf16, tag="encT")
                nc.sync.dma_start(out=encT_b,
                                  in_=encT[:, :, b, :].rearrange("dc p s -> p dc s"))
                encN_b = encN_pool.tile([128, NSC, D], bf16, tag="encN")
                nc.sync.dma_start(out=encN_b,
                                  in_=encN[b].rearrange("sc p d -> p sc d"))
                encN_bs.append(encN_b)
                for sb in range(NSB):
                    s0 = sb * SBLK
                    hT = hT_pool.tile([128, DC, SBLK], bf16, tag="hT")
                    for hc in range(DC):
                        ph = ph_pool.tile([128, SBLK], f32, tag="ph")
                        for dc in range(DC):
                            nc.tensor.matmul(
                                ph,
                                lhsT=w1e_sb[:, dc, hc * 128:(hc + 1) * 128],
                                rhs=encT_b[:, dc, s0:s0 + SBLK],
                                start=(dc == 0), stop=(dc == DC - 1))
                        nc.scalar.activation(out=hT[:, hc, :], in_=ph,
                                             func=AF.Tanh,
                                             bias=pdb1[:, hc, b:b + 1],
                                             scale=1.0)
                    sc_ps = misc_ps.tile([1, SBLK], f32, tag="misc")
                    for hc in range(DC):
                        nc.tensor.matmul(sc_ps,
                                         lhsT=w2_sb[:, hc, :],
                                         rhs=hT[:, hc, :],
                                         start=(hc == 0), stop=(hc == DC - 1))
                    nc.vector.tensor_copy(
                        out=scores_g[32 * bi:32 * bi + 1, s0:s0 + SBLK],
                        in_=sc_ps)

            negmx = small.tile([128, 1], f32, tag="negmx")
            nc.vector.tensor_reduce(out=negmx, in_=scores_g, axis=AX.X,
                                    op=OP.max, negate=True)
            alpha_e = sg_pool.tile([128, S], f32, tag="alpha_e")
            den = small.tile([128, 1], f32, tag="den")
            nc.scalar.activation(out=alpha_e, in_=scores_g, func=AF.Exp,
                                 bias=negmx, scale=1.0, accum_out=den)
            rden = small.tile([128, 1], f32, tag="rden")
            nc.vector.reciprocal(out=rden, in_=den)
            alpha_n = sg_pool.tile([128, S], bf16, tag="alpha_n")
            nc.vector.tensor_scalar(out=alpha_n, in0=alpha_e, scalar1=rden,
                                    scalar2=None, op0=OP.mult)

            alphaT = at_pool.tile([128, NSC, GB], bf16, tag="alphaT")
            for sc in range(NSC):
                tr_ps = misc_ps.tile([128, 128], bf16, tag="misc")
                nc.tensor.transpose(tr_ps,
                                    alpha_n[:, sc * 128:(sc + 1) * 128],
                                    ident_sb)
                # group rows sat at partitions 32*bi -> columns 32*bi after
                # the transpose; gather them into a dense [128, GB] tile.
                nc.vector.tensor_copy(
                    out=alphaT[:, sc, :],
                    in_=tr_ps.rearrange("p (g r) -> p g r", g=GB)[:, :, 0])

            ctx_g = ctxg_pool.tile([128, D], f32, tag="ctxg")
            for bi in range(GB):
                ctx_ps = misc_ps.tile([1, D], f32, tag="misc")
                for sc in range(NSC):
                    nc.tensor.matmul(ctx_ps,
                                     lhsT=alphaT[:, sc, bi:bi + 1],
                                     rhs=encN_bs[bi][:, sc, :],
                                     start=(sc == 0), stop=(sc == NSC - 1))
                nc.vector.tensor_copy(out=ctx_g[32 * bi:32 * bi + 1, :],
                                      in_=ctx_ps)
                nc.sync.dma_start(out=out[g * GB + bi:g * GB + bi + 1, :],
                                  in_=ctx_g[32 * bi:32 * bi + 1, :])

    nc.compile()
    return nc


def _get_nc():
    if "nc" not in _CACHE:
        _CACHE["nc"] = _build()
    return _CACHE["nc"]


def _prep_in_maps(inputs):
    dec = np.asarray(inputs["decoder_hidden"], dtype=np.float32)
    enc = np.asarray(inputs["encoder_outputs"], dtype=np.float32)
    W1 = np.asarray(inputs["W1"], dtype=np.float32)
    b1 = np.asarray(inputs["b1"], dtype=np.float32)
    W2 = np.asarray(inputs["W2"], dtype=np.float32)

    w1eT = np.ascontiguousarray(W1[:, D:].T).reshape(DC, 128, D).astype(_NPBF)
    w1dT = np.ascontiguousarray(W1[:, :D].T).reshape(DC, 128, D).astype(_NPBF)
    b1c = np.ascontiguousarray(b1).reshape(DC, 128, 1).astype(np.float32)
    w2c = np.ascontiguousarray(W2[0]).reshape(DC, 128, 1).astype(_NPBF)
    ident = np.eye(128, dtype=_NPBF)

    in_maps = []
    for c in range(N_CORES):
        sl = slice(c * B_LOC, (c + 1) * B_LOC)
        enc_c = enc[sl]                                  # [16, 1024, 512]
        encT_c = np.ascontiguousarray(enc_c.transpose(2, 0, 1)) \
            .reshape(DC, 128, B_LOC, S).astype(_NPBF)
        encN_c = np.ascontiguousarray(enc_c.reshape(B_LOC, NSC, 128, D)) \
            .astype(_NPBF)
        decT_c = np.ascontiguousarray(dec[sl].T).reshape(DC, 128, B_LOC) \
            .astype(_NPBF)
        in_maps.append({
            "encT": encT_c, "encN": encN_c, "w1eT": w1eT, "w1dT": w1dT,
            "decT": decT_c, "b1c": b1c, "w2c": w2c, "ident": ident,
        })
    return in_maps


def _run(inputs, trace=False, **kw):
    from concourse.bass_utils import run_bass_kernel_spmd
    nc = _get_nc()
    in_maps = _prep_in_maps(inputs)
    res = run_bass_kernel_spmd(nc, in_maps, core_ids=list(range(N_CORES)),
                               trace=trace, **kw)
    outs = [res.results[i]["out"] for i in range(N_CORES)]
    full = np.concatenate(outs, axis=0).astype(np.float32)
    return full, res


def kernel(**inputs) -> np.ndarray:
    full, _ = _run(inputs, trace=False)
    return full
